# revision 3
# baseline (speedup 1.0000x reference)
"""Trainium2 Bass kernel for nn_DomainMapper (segment_reduce + tiny MLP).

Computation (matches the reference):
    sums[s]   = sum of x rows with label s          [32, 640]
    counts[s] = number of rows with label s         [32]
    feats     = sums / counts
    h         = relu(feats @ W1 + b1)               [32, 256]
    logits    = h @ W2 + b2                         [32, 32]
    probs     = softmax(logits, axis=-1)
    returns (probs, arange(32))

Strategy: data-parallel over 8 NeuronCores. Each core reads its 16384-row
shard of x once (the memory-bound part), computes local segment sums via
one-hot matmuls on the TensorEngine (general for any label distribution),
AllReduces the [32, 641] partials (sums ++ counts), then every core runs the
tiny MLP + softmax replicated; core 0's output is used.
"""

import numpy as np

import concourse.bass as bass
import concourse.bacc as bacc
import concourse.mybir as mybir
import concourse.tile as tile
from concourse.bass_utils import run_bass_kernel_spmd

N_CORES = 8
N, D, H, S = 131072, 640, 256, 32
ROWS = N // N_CORES  # rows per core
P = 128              # partitions / rows per matmul subtile
KC1 = D // P         # 5 contraction chunks for feats @ W1
KC2 = H // P         # 2 contraction chunks for h @ W2

PROFILE = False
LAST_EXEC_NS = None
LAST_RESULTS = None

_nc_cache = {}


def build_nc(rows=ROWS, g=8):
    """Build + compile the per-core Bass graph. `rows` must be a multiple of
    128*g. One graph is shared SPMD across all 8 cores."""
    T = rows // P            # number of 128-row subtiles
    assert T % g == 0
    f32 = mybir.dt.float32
    AF = mybir.ActivationFunctionType
    ALU = mybir.AluOpType

    nc = bacc.Bacc("TRN2", target_bir_lowering=False, debug=False,
                   num_devices=N_CORES)

    x = nc.dram_tensor("x", [rows, D], f32, kind="ExternalInput").ap()
    labt = nc.dram_tensor("labt", [P, T], f32, kind="ExternalInput").ap()
    w1 = nc.dram_tensor("w1", [D, H], f32, kind="ExternalInput").ap()
    b1 = nc.dram_tensor("b1", [1, H], f32, kind="ExternalInput").ap()
    w2 = nc.dram_tensor("w2", [H, S], f32, kind="ExternalInput").ap()
    b2 = nc.dram_tensor("b2", [1, S], f32, kind="ExternalInput").ap()
    iota = nc.dram_tensor("iota", [P, S], f32, kind="ExternalInput").ap()
    ident = nc.dram_tensor("ident", [S, S], f32, kind="ExternalInput").ap()
    probs = nc.dram_tensor("probs", [S, S], f32, kind="ExternalOutput").ap()

    with tile.TileContext(nc) as tc:
        with (
            tc.tile_pool(name="const", bufs=1) as cpool,
            tc.tile_pool(name="xload", bufs=4) as xpool,
            tc.tile_pool(name="oh", bufs=4) as ohpool,
            tc.tile_pool(name="acc", bufs=1, space=bass.MemorySpace.PSUM) as apool,
            tc.tile_pool(name="mm", bufs=1, space=bass.MemorySpace.PSUM) as mpool,
            tc.tile_pool(name="small", bufs=1) as spool,
            tc.tile_pool(name="dram", bufs=1, space=bass.MemorySpace.DRAM) as dpool,
        ):
            # ---- constants / weights ----
            labt_sb = cpool.tile([P, T], f32)
            nc.sync.dma_start(labt_sb[:], labt[:])
            iota_sb = cpool.tile([P, S], f32)
            nc.sync.dma_start(iota_sb[:], iota[:])
            ident_sb = cpool.tile([S, S], f32)
            nc.sync.dma_start(ident_sb[:], ident[:])
            w1_sb = cpool.tile([P, KC1, H], f32)
            nc.sync.dma_start(w1_sb[:], w1.rearrange("(k p) h -> p k h", p=P))
            w2_sb = cpool.tile([P, KC2, S], f32)
            nc.sync.dma_start(w2_sb[:], w2.rearrange("(k p) s -> p k s", p=P))
            b1_sb = cpool.tile([1, H], f32)
            nc.sync.dma_start(b1_sb[:], b1[:])
            b2_sb = cpool.tile([1, S], f32)
            nc.sync.dma_start(b2_sb[:], b2[:])
            ones_col = cpool.tile([P, 1], f32)
            nc.vector.memset(ones_col[:], 1.0)
            ones_row = cpool.tile([1, S], f32)
            nc.vector.memset(ones_row[:], 1.0)

            # ---- local segment sums: psum += onehot.T @ x_subtile ----
            psumA = apool.tile([S, 512], f32)   # x cols 0:512
            psumB = apool.tile([S, D - 512], f32)  # x cols 512:640
            psumC = apool.tile([S, 1], f32)     # counts

            xg = x.rearrange("(gg j p) d -> gg p j d", j=g, p=P)
            for gi in range(T // g):
                xt = xpool.tile([P, g, D], f32)
                nc.sync.dma_start(xt[:], xg[gi])
                for j in range(g):
                    t = gi * g + j
                    oh = ohpool.tile([P, S], f32)
                    nc.vector.tensor_scalar(
                        oh[:], iota_sb[:], labt_sb[:, t:t + 1], None, ALU.is_equal)
                    fs = dict(start=(t == 0), stop=(t == T - 1))
                    nc.tensor.matmul(psumA[:], oh[:], xt[:, j, 0:512], **fs)
                    nc.tensor.matmul(psumB[:], oh[:], xt[:, j, 512:D], **fs)
                    nc.tensor.matmul(psumC[:], oh[:], ones_col[:], **fs)

            # ---- AllReduce partial sums+counts across the 8 cores ----
            part = spool.tile([S, D + 1], f32)
            nc.scalar.copy(part[:, 0:512], psumA[:])
            nc.scalar.copy(part[:, 512:D], psumB[:])
            nc.scalar.copy(part[:, D:D + 1], psumC[:])
            cc_in = dpool.tile([S, D + 1], f32)
            cc_out = dpool.tile([S, D + 1], f32, addr_space="Shared")
            nc.sync.dma_start(cc_in[:], part[:])
            nc.gpsimd.collective_compute(
                "AllReduce", ALU.add,
                replica_groups=[list(range(N_CORES))],
                ins=[cc_in.opt()], outs=[cc_out.opt()])
            tot = spool.tile([S, D + 1], f32)
            nc.sync.dma_start(tot[:], cc_out[:])

            # ---- feats = sums / counts ----
            recip = spool.tile([S, 1], f32)
            nc.vector.reciprocal(recip[:], tot[:, D:D + 1])
            feats = spool.tile([S, D], f32)
            nc.scalar.mul(feats[:], tot[:, 0:D], recip[:, 0:1])

            # ---- h = relu(feats @ W1 + b1) ----
            ftT = spool.tile([P, KC1, S], f32)
            for k in range(KC1):
                pt = mpool.tile([P, S], f32)
                nc.tensor.transpose(pt[:], feats[:, k * P:(k + 1) * P], ident_sb[:])
                nc.scalar.copy(ftT[:, k, :], pt[:])
            h_ps = mpool.tile([S, H], f32)
            for k in range(KC1):
                nc.tensor.matmul(h_ps[:], ftT[:, k, :], w1_sb[:, k, :],
                                 start=(k == 0), stop=False)
            nc.tensor.matmul(h_ps[:], ones_row[:1, :], b1_sb[:1, :],
                             start=False, stop=True)
            h_sb = spool.tile([S, H], f32)
            nc.scalar.activation(h_sb[:], h_ps[:], AF.Relu)

            # ---- logits = h @ W2 + b2 ----
            hT = spool.tile([P, KC2, S], f32)
            for k in range(KC2):
                pt2 = mpool.tile([P, S], f32, tag="pt")
                nc.tensor.transpose(pt2[:], h_sb[:, k * P:(k + 1) * P], ident_sb[:])
                nc.scalar.copy(hT[:, k, :], pt2[:])
            l_ps = mpool.tile([S, S], f32)
            for k in range(KC2):
                nc.tensor.matmul(l_ps[:], hT[:, k, :], w2_sb[:, k, :],
                                 start=(k == 0), stop=False)
            nc.tensor.matmul(l_ps[:], ones_row[:1, :], b2_sb[:1, :],
                             start=False, stop=True)

            # ---- softmax over the free dim ----
            negmax = spool.tile([S, 1], f32)
            nc.vector.tensor_reduce(negmax[:], l_ps[:], axis=mybir.AxisListType.X,
                                    op=ALU.max, negate=True)
            e = spool.tile([S, S], f32)
            se = spool.tile([S, 1], f32)
            nc.scalar.activation(e[:], l_ps[:], AF.Exp, bias=negmax[:, 0:1],
                                 accum_out=se[:])
            rse = spool.tile([S, 1], f32)
            nc.vector.reciprocal(rse[:], se[:])
            pr = spool.tile([S, S], f32)
            nc.scalar.mul(pr[:], e[:], rse[:, 0:1])
            nc.sync.dma_start(probs[:], pr[:])

    nc.compile()
    return nc


def _get_nc(rows=ROWS, g=8):
    key = (rows, g)
    if key not in _nc_cache:
        _nc_cache[key] = build_nc(rows, g)
    return _nc_cache[key]


def make_in_maps(x, subject_labels, W1, b1, W2, b2, rows=ROWS):
    """Shard the full inputs into per-core input maps (host side)."""
    n_cores = x.shape[0] // rows
    T = rows // P
    xs = np.ascontiguousarray(np.asarray(x, dtype=np.float32)).reshape(
        n_cores, rows, D)
    lab = np.asarray(subject_labels).astype(np.float32).reshape(n_cores, T, P)
    labt = np.ascontiguousarray(lab.transpose(0, 2, 1))  # [c, p, t]
    w1 = np.ascontiguousarray(np.asarray(W1, dtype=np.float32))
    b1r = np.ascontiguousarray(np.asarray(b1, dtype=np.float32).reshape(1, H))
    w2 = np.ascontiguousarray(np.asarray(W2, dtype=np.float32))
    b2r = np.ascontiguousarray(np.asarray(b2, dtype=np.float32).reshape(1, S))
    iota = np.ascontiguousarray(
        np.tile(np.arange(S, dtype=np.float32), (P, 1)))
    ident = np.eye(S, dtype=np.float32)
    return [
        dict(x=xs[c], labt=labt[c], w1=w1, b1=b1r, w2=w2, b2=b2r,
             iota=iota, ident=ident)
        for c in range(n_cores)
    ]


def kernel(x, subject_labels, W1, b1, W2, b2):
    global LAST_EXEC_NS, LAST_RESULTS
    x = np.asarray(x)
    subject_labels = np.asarray(subject_labels)
    nc = _get_nc()
    in_maps = make_in_maps(x, subject_labels, W1, b1, W2, b2)
    kwargs = {}
    if PROFILE:
        kwargs = dict(trace=True)
    res = run_bass_kernel_spmd(nc, in_maps, core_ids=list(range(N_CORES)),
                               **kwargs)
    LAST_EXEC_NS = res.exec_time_ns
    LAST_RESULTS = res
    probs = np.asarray(res.results[0]["probs"], dtype=np.float32)
    unique_ids = np.arange(S, dtype=subject_labels.dtype)
    return probs, unique_ids
